# revision 8
# baseline (speedup 1.0000x reference)
"""Trainium2 Bass kernel for nn_DeChunkLayer.

Per batch row (one NeuronCore each, pure data parallel):
  1. gate[c]: boundary-sorted clipped probabilities (host, tiny).
  2. EMA linear recurrence over chunks h_c = (1-g_c) h_{c-1} + g_c x_c as a
     blocked lower-triangular matmul "scan": for each 128-chunk block t,
       ema_t = L_t @ X_t + L2_t @ X_{t-1}
     with coefficients host-computed in f64 log space. The one-block (128
     chunk) lookback is exact to fp16 resolution because the decay product
     over 128 chunks underflows far below fp32 (host-verified bound).
  3. Dechunk out[s] = ema[cid[s]] as one-hot selection matmuls per 128-token
     block. Each token block uses a single 32-aligned 128-chunk window of
     ema whenever the cross-core union of its chunk ids spans < 128 chunks
     (the common case); windows that are not 128-aligned are stitched from
     the two adjacent ema blocks by cheap f16 SBUF copies on the otherwise
     idle GpSimd engine. Blocks whose union span exceeds the window fall
     back to 2-3 aligned-block matmuls accumulated in PSUM.
     Selection matrices are built on-device: is_equal of u8 local chunk ids
     (cid - window_base, host-packed) against a u8 iota column.

All matmul operands are fp16 (PSUM accumulates fp32): values are O(5) so
fp16 keeps abs err ~4e-3 (rel ~3.5e-4) while running the PE at full rate.
"""

import math

import numpy as np

import concourse.bacc as bacc
import concourse.mybir as mybir
from concourse import tile
from concourse.bass_utils import run_bass_kernel_spmd

B, SEQ, MAXC, DIM = 8, 4096, 2048, 1024
BLK = 128
NTB = SEQ // BLK  # 32 token blocks
NCORES = 8
F32 = mybir.dt.float32
F16 = mybir.dt.float16
U8 = mybir.dt.uint8
# output staging group sizes (token blocks per out DMA); tapered tail so the
# final DMA after the last matmul is small
GRPS = [1, 1, 2, 2, 4, 4, 4, 4, 4, 2, 2, 1, 1]
assert sum(GRPS) == NTB

# engine partition-access limit by base partition (BIR verifier rule)
_MAXCNT = {0: 128, 32: 32, 64: 64, 96: 32}


def _legal_pieces(src0, dst0, cnt, blk):
    """Split a partition-range copy into verifier-legal sub-copies."""
    out = []
    q = 0
    while q < cnt:
        s, d = src0 + q, dst0 + q
        c = min(cnt - q, _MAXCNT[s % BLK], _MAXCNT[d % BLK])
        out.append((s, d, c, blk))
        q += c
    return out


def _preprocess(chunk_states, boundary_mask, boundary_prob):
    """Host-side index/gate math.

    Returns (in_maps, NBLK, wins, prod_ws) where wins[tb] is the list of
    32-aligned window starts for token block tb and prod_ws the ordered
    list of non-128-aligned windows that must be stitched on device.
    """
    chunk_states = np.asarray(chunk_states, dtype=np.float32)
    boundary_mask = np.asarray(boundary_mask)
    boundary_prob = np.asarray(boundary_prob, dtype=np.float32)

    p_full = np.clip(boundary_prob[..., -1], np.float32(1e-4), np.float32(1.0 - 1e-4))
    token_idx = np.arange(SEQ)[None, :] + (~boundary_mask).astype(np.int32) * SEQ
    order = np.argsort(token_idx, axis=1, kind="stable")
    gate = np.take_along_axis(p_full, order[:, :MAXC], axis=1)  # [B, C]

    cid = np.cumsum(boundary_mask.astype(np.int32), axis=1) - 1  # [B, S]
    cid = np.clip(cid, 0, MAXC - 1)
    n_used = int(cid.max()) + 1
    NBLK = max(1, math.ceil(n_used / BLK))
    CU = NBLK * BLK

    g = gate[:, :CU].astype(np.float64)
    a = 1.0 - g
    S = np.cumsum(np.log(a), axis=1)  # [B, CU] global log-decay prefix

    # one-block lookback must cover everything older than the previous block
    for t in range(2, NBLK):
        j0 = (t - 1) * BLK - 1
        if np.any(S[:, t * BLK] - S[:, j0] > -18.0):
            raise RuntimeError("128-chunk lookback decay bound violated")

    ii = np.arange(BLK)[:, None]
    jj = np.arange(BLK)[None, :]
    Sb = S.reshape(B, NBLK, BLK)
    # main (within-block) coefficients: L[b,t,i,j] = g_j exp(S_i - S_j), i>=j
    Lf = np.where(
        ii[None, None] >= jj[None, None],
        np.exp(Sb[:, :, :, None] - Sb[:, :, None, :])
        * g.reshape(B, NBLK, 1, BLK),
        0.0,
    )
    # lhsT layout: lt[b, j, t*128 + i]
    LT_sb = np.ascontiguousarray(
        Lf.transpose(0, 3, 1, 2).reshape(B, BLK, NBLK * BLK).astype(np.float16)
    )

    # full-block lookback: chunk (t-1)*128+j feeding out chunk t*128+i
    lt2_sb = np.zeros((B, BLK, NBLK * BLK), dtype=np.float16)
    for t in range(1, NBLK):
        Sout = S[:, t * BLK:(t + 1) * BLK]  # [B, 128]
        Sin = S[:, (t - 1) * BLK:t * BLK]  # [B, 128]
        gin = g[:, (t - 1) * BLK:t * BLK]
        Lb = np.exp(Sout[:, None, :] - Sin[:, :, None]) * gin[:, :, None]
        lt2_sb[:, :, t * BLK:(t + 1) * BLK] = Lb.astype(np.float16)

    # dechunk windows: per token block one 32-aligned 128-chunk window when
    # the union (all cores) span allows it, else aligned 128-blocks.
    # Window stitch cost rises with misalignment (128-aligned: free, 64: two
    # legal partition copies, 32/96: four), so pick the cheapest feasible
    # start and reuse windows across token blocks.
    cidr = cid.reshape(B, NTB, BLK)
    lo = cidr[:, :, 0].min(axis=0)
    hi = cidr[:, :, -1].max(axis=0)
    wins = []
    chosen = set()
    for tb in range(NTB):
        l, h = int(lo[tb]), int(hi[tb])
        w_hi = (l // 32) * 32          # largest feasible start
        w_lo = max(0, -(-(h - 127) // 32) * 32)  # smallest feasible start
        if w_lo > w_hi:
            t0, t1 = l // BLK, h // BLK
            wins.append([BLK * t for t in range(t0, t1 + 1)])
            continue
        cands = range(w_lo, w_hi + 1, 32)
        pick = None
        for w in cands:  # reuse an already-chosen window if possible
            if w in chosen:
                pick = w
                break
        if pick is None:
            for align in (128, 64, 32):
                for w in cands:
                    if w % align == 0:
                        pick = w
                        break
                if pick is not None:
                    break
        chosen.add(pick)
        wins.append([pick])
    prod_ws = sorted({w for ws in wins for w in ws if w % BLK != 0})

    # u8 local chunk ids: cid - first window base of the token block
    w0 = np.array([ws[0] for ws in wins], dtype=np.int32)  # [NTB]
    cid_local = cid - np.repeat(w0, BLK)[None, :]
    assert cid_local.min() >= 0 and cid_local.max() < 256, (
        cid_local.min(), cid_local.max())
    cid8 = cid_local.astype(np.uint8)

    in_maps = []
    for b in range(B):
        in_maps.append(
            {
                "x": np.ascontiguousarray(chunk_states[b, :CU].astype(np.float16)),
                "lt": LT_sb[b],
                "lt2": np.ascontiguousarray(lt2_sb[b]),
                "cidb": np.ascontiguousarray(
                    np.broadcast_to(cid8[b][None, :], (BLK, SEQ))
                ),
            }
        )
    return in_maps, NBLK, wins, prod_ws


def _build_nc(NBLK, wins, prod_ws):
    nc = bacc.Bacc("TRN2", target_bir_lowering=False, debug=False, num_devices=8)
    x = nc.dram_tensor("x", [NBLK * BLK, DIM], F16, kind="ExternalInput")
    lt = nc.dram_tensor("lt", [BLK, NBLK * BLK], F16, kind="ExternalInput")
    lt2 = nc.dram_tensor("lt2", [BLK, NBLK * BLK], F16, kind="ExternalInput")
    cidb = nc.dram_tensor("cidb", [BLK, SEQ], U8, kind="ExternalInput")
    out = nc.dram_tensor("out", [SEQ, DIM], F16, kind="ExternalOutput")

    # per-window production step: w -> scan block whose CAST unblocks it
    def wneed(w):
        return w // BLK if w % BLK == 0 else min(w // BLK + 1, NBLK - 1)

    need_zblk = any(w // BLK == NBLK - 1 and w % BLK for w in prod_ws)
    prod_at = {}  # scan block t -> list of windows to stitch after its CAST
    for w in prod_ws:
        prod_at.setdefault(wneed(w), []).append(w)

    with tile.TileContext(nc) as tc:
        with (
            tc.tile_pool(name="const", bufs=1) as const_pool,
            tc.tile_pool(name="selp", bufs=12) as selpool,
            tc.tile_pool(name="outp", bufs=4) as outpool,
            tc.tile_pool(name="ps_scan", bufs=2, space="PSUM") as ps_scan,
            tc.tile_pool(name="ps_out", bufs=2, space="PSUM") as ps_out,
        ):
            # load order: scan weights + x first (critical path) on the sync
            # ring; index data + lookback weights on the scalar ring.
            lt_sb = const_pool.tile([BLK, NBLK * BLK], F16, tag="lt")
            nc.sync.dma_start(lt_sb[:], lt[:])
            x_sb = const_pool.tile([BLK, NBLK * DIM], F16, tag="x")
            # stage x in pieces so early scan blocks unblock fast
            xcuts = sorted({min(1, NBLK), min(2, NBLK), min(3, NBLK),
                            min(5, NBLK), min(7, NBLK), NBLK})
            c_prev = 0
            for c1 in xcuts:
                if c1 <= c_prev:
                    continue
                nc.sync.dma_start(
                    x_sb[:, c_prev * DIM:c1 * DIM],
                    x[c_prev * BLK:c1 * BLK, :].rearrange(
                        "(t p) d -> p t d", p=BLK),
                )
                c_prev = c1
            cidb_sb = const_pool.tile([BLK, SEQ], U8, tag="cidb")
            nc.scalar.dma_start(cidb_sb[:, :2048], cidb[:, :2048])
            nc.scalar.dma_start(cidb_sb[:, 2048:], cidb[:, 2048:])
            lt2_sb = const_pool.tile([BLK, NBLK * BLK], F16, tag="lt2")
            nc.scalar.dma_start(lt2_sb[:], lt2[:])

            # per-partition compare values: col k holds p + 128k (f32 — the
            # tensor_scalar scalar operand must be float32 for is_equal)
            iota8 = const_pool.tile([BLK, 2], F32, tag="iota8")
            nc.gpsimd.iota(iota8[:], [[BLK, 2]], channel_multiplier=1,
                           allow_small_or_imprecise_dtypes=True)

            # ema: one extra zeroed block so window stitches that read past
            # the last scan block pull zeros (sel never selects them)
            ema = const_pool.tile([BLK, (NBLK + 1) * DIM], F16, tag="ema")
            if need_zblk:
                nc.vector.memset(ema[:, NBLK * DIM:], 0.0)
            win_sb = {}
            for w in prod_ws:
                win_sb[w] = const_pool.tile([BLK, DIM], F16, tag=f"win{w}",
                                            name=f"win_{w}")

            # PE warmup: zero-weight matmuls accumulating into block 0's
            # psum (add 0, cannot be dead-code-eliminated). ~3us of PE
            # activity releases the HAM clock throttle before real work.
            zw = const_pool.tile([BLK, BLK], F16, tag="zw")
            nc.vector.memset(zw[:], 0.0)
            zx = const_pool.tile([BLK, 512], F16, tag="zx")
            nc.vector.memset(zx[:], 0.0)
            ps0 = ps_scan.tile([BLK, DIM], F32, tag="ps")
            for k in range(4):
                for h in range(2):
                    nc.tensor.matmul(
                        ps0[:, h * 512:(h + 1) * 512], lhsT=zw[:], rhs=zx[:],
                        start=(k == 0), stop=False,
                    )

            # ---- dechunk emitter (interleaved with the scan so the PE
            # queue never stalls behind scan blocks waiting on late DMAs) ---
            state = {"tb": 0, "gi": 0}

            def emit_group(grp):
                gi = state["gi"]
                og = outpool.tile([BLK, grp * DIM], F16, tag=f"og{grp}",
                                  name=f"og_{gi}")
                for i in range(grp):
                    tb = state["tb"]
                    ws = wins[tb]
                    w0 = ws[0]
                    sels = []
                    for k, w in enumerate(ws):
                        sel = selpool.tile([BLK, BLK], F16, tag="sel",
                                           name=f"sel_{tb}_{k}")
                        nc.vector.tensor_scalar(
                            out=sel[:],
                            in0=cidb_sb[:, tb * BLK:(tb + 1) * BLK],
                            scalar1=iota8[:, k:k + 1],
                            scalar2=None,
                            op0=mybir.AluOpType.is_equal,
                        )
                        sels.append((sel, w))
                    po = ps_out.tile([BLK, DIM], F32, tag="po",
                                     name=f"po_{tb}")
                    for wi, (sel, w) in enumerate(sels):
                        for h in range(2):
                            if w % BLK == 0:
                                rsrc = ema[:, (w // BLK) * DIM + h * 512:
                                           (w // BLK) * DIM + (h + 1) * 512]
                            else:
                                rsrc = win_sb[w][:, h * 512:(h + 1) * 512]
                            nc.tensor.matmul(
                                po[:, h * 512:(h + 1) * 512],
                                lhsT=sel[:],
                                rhs=rsrc,
                                start=(wi == 0),
                                stop=(wi == len(sels) - 1),
                            )
                    dst = og[:, i * DIM:(i + 1) * DIM]
                    # 3/8 of the psum->og casts on DVE, 5/8 on ACT (DVE also
                    # builds sels; ACT only splits the ema casts)
                    if (tb % 8) in (0, 3, 6):
                        nc.vector.tensor_copy(out=dst, in_=po[:])
                    else:
                        nc.scalar.copy(out=dst, in_=po[:])
                    state["tb"] = tb + 1
                tb0 = state["tb"] - grp
                dma_eng = nc.scalar if (gi % 2) == 0 else nc.sync
                dma_eng.dma_start(
                    out[tb0 * BLK:state["tb"] * BLK, :].rearrange(
                        "(i p) d -> p i d", p=BLK
                    ),
                    og[:].rearrange("p (i d) -> p i d", d=DIM),
                )
                state["gi"] = gi + 1

            # a group is ready once the last scan block it depends on (via
            # direct windows or stitched ones) is written
            group_need = []
            tb = 0
            for grp in GRPS:
                group_need.append(
                    max(wneed(w) for t in range(tb, tb + grp)
                        for w in wins[t]))
                tb += grp

            # ---- blocked matmul scan over chunk blocks ----
            for t in range(NBLK):
                ps = ps0 if t == 0 else ps_scan.tile([BLK, DIM], F32, tag="ps")
                for h in range(2):
                    sl = slice(h * 512, (h + 1) * 512)
                    xsl = slice(t * DIM + h * 512, t * DIM + (h + 1) * 512)
                    nc.tensor.matmul(
                        ps[:, sl],
                        lhsT=lt_sb[:, t * BLK:(t + 1) * BLK],
                        rhs=x_sb[:, xsl],
                        start=(t != 0),
                        stop=(t == 0),
                    )
                    if t > 0:
                        lsl = slice((t - 1) * DIM + h * 512,
                                    (t - 1) * DIM + (h + 1) * 512)
                        nc.tensor.matmul(
                            ps[:, sl],
                            lhsT=lt2_sb[:, t * BLK:(t + 1) * BLK],
                            rhs=x_sb[:, lsl],
                            start=False,
                            stop=True,
                        )
                # psum -> fp16 ema, split across DVE and ACT
                nc.vector.tensor_copy(
                    out=ema[:, t * DIM:t * DIM + 512], in_=ps[:, :512]
                )
                nc.scalar.copy(
                    out=ema[:, t * DIM + 512:(t + 1) * DIM], in_=ps[:, 512:]
                )
                # stitch any 32-aligned windows unblocked by this block's
                # CAST on the otherwise idle GpSimd engine. Engine partition
                # accesses are limited by base (0:128, 32:32, 64:64, 96:32),
                # so split each piece into legal sub-copies.
                for w in prod_at.get(t, ()):
                    t0, off = w // BLK, w % BLK
                    pieces = (_legal_pieces(off, 0, BLK - off, t0)
                              + _legal_pieces(0, BLK - off, off, t0 + 1))
                    for s, d, c, tt in pieces:
                        nc.gpsimd.tensor_copy(
                            out=win_sb[w][d:d + c, :],
                            in_=ema[s:s + c, tt * DIM:(tt + 1) * DIM],
                        )
                while (state["gi"] < len(GRPS)
                       and group_need[state["gi"]] <= t):
                    emit_group(GRPS[state["gi"]])

            while state["gi"] < len(GRPS):
                emit_group(GRPS[state["gi"]])

    nc.finalize()
    return nc


def _run(in_maps, NBLK, wins, prod_ws):
    nc = _build_nc(NBLK, wins, prod_ws)
    res = run_bass_kernel_spmd(nc, in_maps, core_ids=list(range(NCORES)))
    return np.stack(
        [res.results[i]["out"].astype(np.float32) for i in range(NCORES)], axis=0
    )


def kernel(chunk_states, boundary_mask, boundary_prob):
    in_maps, NBLK, wins, prod_ws = _preprocess(
        chunk_states, boundary_mask, boundary_prob
    )
    last_err = None
    for _ in range(3):  # retry transient accelerator failures
        try:
            return _run(in_maps, NBLK, wins, prod_ws)
        except Exception as e:  # noqa: BLE001
            last_err = e
            try:
                import jax

                jax.clear_caches()
            except Exception:  # noqa: BLE001
                pass
    raise last_err


# revision 12
# speedup vs baseline: 3.8165x; 3.8165x over previous
"""Trainium2 Bass kernel for nn_DeChunkLayer.

Per batch row (one NeuronCore each, pure data parallel):
  1. gate[c]: boundary-sorted clipped probabilities (host, tiny).
  2. EMA linear recurrence over chunks h_c = (1-g_c) h_{c-1} + g_c x_c as a
     blocked lower-triangular matmul "scan": for each 128-chunk block t,
       ema_t = L_t @ X_t + L2_t @ X_{t-1}
     with coefficients host-computed in f64 log space. The one-block (128
     chunk) lookback is exact to fp16 resolution because the decay product
     over 128 chunks underflows far below fp32 (host-verified bound).
  3. Dechunk out[s] = ema[cid[s]] as one-hot selection matmuls per 128-token
     block. Each token block uses a single 32-aligned 128-chunk window of
     ema whenever the cross-core union of its chunk ids spans < 128 chunks
     (the common case); windows that are not 128-aligned are stitched from
     the two adjacent ema blocks by cheap f16 SBUF copies on the otherwise
     idle GpSimd engine. Blocks whose union span exceeds the window fall
     back to 2-3 aligned-block matmuls accumulated in PSUM.
     Selection matrices are built on-device: is_equal of u8 local chunk ids
     (cid - window_base, host-packed) against a u8 iota column.

All matmul operands are fp16 (PSUM accumulates fp32): values are O(5) so
fp16 keeps abs err ~4e-3 (rel ~3.5e-4) while running the PE at full rate.
"""

import math

import numpy as np

import concourse.bacc as bacc
import concourse.mybir as mybir
from concourse import tile
from concourse.bass_utils import run_bass_kernel_spmd

B, SEQ, MAXC, DIM = 8, 4096, 2048, 1024
BLK = 128
NTB = SEQ // BLK  # 32 token blocks
NCORES = 8
F32 = mybir.dt.float32
F16 = mybir.dt.float16
U8 = mybir.dt.uint8
# output staging group sizes (token blocks per out DMA); tapered tail so the
# final DMA after the last matmul is small
GRPS = [1, 1, 2, 2, 4, 4, 4, 4, 4, 2, 2, 1, 1]
assert sum(GRPS) == NTB

# engine partition-access limit by base partition (BIR verifier rule)
_MAXCNT = {0: 128, 32: 32, 64: 64, 96: 32}


def _legal_pieces(src0, dst0, cnt, blk):
    """Split a partition-range copy into verifier-legal sub-copies."""
    out = []
    q = 0
    while q < cnt:
        s, d = src0 + q, dst0 + q
        c = min(cnt - q, _MAXCNT[s % BLK], _MAXCNT[d % BLK])
        out.append((s, d, c, blk))
        q += c
    return out


def _preprocess(chunk_states, boundary_mask, boundary_prob):
    """Host-side index/gate math.

    Returns (in_maps, NBLK, wins, prod_ws) where wins[tb] is the list of
    32-aligned window starts for token block tb and prod_ws the ordered
    list of non-128-aligned windows that must be stitched on device.
    """
    chunk_states = np.asarray(chunk_states, dtype=np.float32)
    boundary_mask = np.asarray(boundary_mask)
    boundary_prob = np.asarray(boundary_prob, dtype=np.float32)

    p_full = np.clip(boundary_prob[..., -1], np.float32(1e-4), np.float32(1.0 - 1e-4))
    token_idx = np.arange(SEQ)[None, :] + (~boundary_mask).astype(np.int32) * SEQ
    order = np.argsort(token_idx, axis=1, kind="stable")
    gate = np.take_along_axis(p_full, order[:, :MAXC], axis=1)  # [B, C]

    cid = np.cumsum(boundary_mask.astype(np.int32), axis=1) - 1  # [B, S]
    cid = np.clip(cid, 0, MAXC - 1)
    n_used = int(cid.max()) + 1
    NBLK = max(1, math.ceil(n_used / BLK))
    CU = NBLK * BLK

    g = gate[:, :CU].astype(np.float64)
    a = 1.0 - g
    S = np.cumsum(np.log(a), axis=1)  # [B, CU] global log-decay prefix

    # one-block lookback must cover everything older than the previous block
    for t in range(2, NBLK):
        j0 = (t - 1) * BLK - 1
        if np.any(S[:, t * BLK] - S[:, j0] > -18.0):
            raise RuntimeError("128-chunk lookback decay bound violated")

    ii = np.arange(BLK)[:, None]
    jj = np.arange(BLK)[None, :]
    Sb = S.reshape(B, NBLK, BLK)
    # main (within-block) coefficients: L[b,t,i,j] = g_j exp(S_i - S_j), i>=j
    Lf = np.where(
        ii[None, None] >= jj[None, None],
        np.exp(Sb[:, :, :, None] - Sb[:, :, None, :])
        * g.reshape(B, NBLK, 1, BLK),
        0.0,
    )
    # lhsT layout: lt[b, j, t*128 + i]
    LT_sb = np.ascontiguousarray(
        Lf.transpose(0, 3, 1, 2).reshape(B, BLK, NBLK * BLK).astype(np.float16)
    )

    # full-block lookback: chunk (t-1)*128+j feeding out chunk t*128+i
    lt2_sb = np.zeros((B, BLK, NBLK * BLK), dtype=np.float16)
    for t in range(1, NBLK):
        Sout = S[:, t * BLK:(t + 1) * BLK]  # [B, 128]
        Sin = S[:, (t - 1) * BLK:t * BLK]  # [B, 128]
        gin = g[:, (t - 1) * BLK:t * BLK]
        Lb = np.exp(Sout[:, None, :] - Sin[:, :, None]) * gin[:, :, None]
        lt2_sb[:, :, t * BLK:(t + 1) * BLK] = Lb.astype(np.float16)

    # dechunk windows: per token block one 64-aligned 128-chunk window when
    # the union (all cores) span allows it, else aligned 128-blocks.
    # 64-offset windows cost two cheap f16 partition copies to stitch;
    # other offsets are not worth it (engine partition-access limits force
    # 4 copies), so those token blocks use 2 aligned-block matmuls instead.
    cidr = cid.reshape(B, NTB, BLK)
    lo = cidr[:, :, 0].min(axis=0)
    hi = cidr[:, :, -1].max(axis=0)
    wins = []
    chosen = set()
    for tb in range(NTB):
        l, h = int(lo[tb]), int(hi[tb])
        w_hi = (l // 64) * 64          # largest feasible start
        w_lo = max(0, -(-(h - 127) // 64) * 64)  # smallest feasible start
        pick = None
        if w_lo <= w_hi:
            cands = range(w_lo, w_hi + 1, 64)
            for w in cands:  # reuse an already-stitched window if possible
                if w in chosen:
                    pick = w
                    break
            if pick is None:
                for align in (128, 64):
                    for w in cands:
                        if w % align == 0:
                            pick = w
                            break
                    if pick is not None:
                        break
        if pick is None:
            t0, t1 = l // BLK, h // BLK
            wins.append([BLK * t for t in range(t0, t1 + 1)])
        else:
            chosen.add(pick)
            wins.append([pick])
    prod_ws = sorted({w for ws in wins for w in ws if w % BLK != 0})

    # f16 local chunk ids: cid - first window base of the token block
    # (small integers, exact in f16; f16 keeps the DVE is_equal at full rate)
    w0 = np.array([ws[0] for ws in wins], dtype=np.int32)  # [NTB]
    cid_local = cid - np.repeat(w0, BLK)[None, :]
    assert cid_local.min() >= 0 and cid_local.max() < 384, (
        cid_local.min(), cid_local.max())
    cidf = cid_local.astype(np.float16)

    in_maps = []
    for b in range(B):
        in_maps.append(
            {
                "x": np.ascontiguousarray(chunk_states[b, :CU].astype(np.float16)),
                "lt": LT_sb[b],
                "lt2": np.ascontiguousarray(lt2_sb[b]),
                "cidb": np.ascontiguousarray(
                    np.broadcast_to(cidf[b][None, :], (BLK, SEQ))
                ),
            }
        )
    return in_maps, NBLK, wins, prod_ws


def _build_nc(NBLK, wins, prod_ws):
    nc = bacc.Bacc("TRN2", target_bir_lowering=False, debug=False, num_devices=8)
    x = nc.dram_tensor("x", [NBLK * BLK, DIM], F16, kind="ExternalInput")
    lt = nc.dram_tensor("lt", [BLK, NBLK * BLK], F16, kind="ExternalInput")
    lt2 = nc.dram_tensor("lt2", [BLK, NBLK * BLK], F16, kind="ExternalInput")
    cidb = nc.dram_tensor("cidb", [BLK, SEQ], F16, kind="ExternalInput")
    out = nc.dram_tensor("out", [SEQ, DIM], F16, kind="ExternalOutput")

    # per-window production step: w -> scan block whose CAST unblocks it
    def wneed(w):
        return w // BLK if w % BLK == 0 else min(w // BLK + 1, NBLK - 1)

    need_zblk = any(w // BLK == NBLK - 1 and w % BLK for w in prod_ws)
    prod_at = {}  # scan block t -> list of windows to stitch after its CAST
    for w in prod_ws:
        prod_at.setdefault(wneed(w), []).append(w)

    with tile.TileContext(nc) as tc:
        with (
            tc.tile_pool(name="const", bufs=1) as const_pool,
            tc.tile_pool(name="selp", bufs=12) as selpool,
            tc.tile_pool(name="outp", bufs=4) as outpool,
            tc.tile_pool(name="ps_scan", bufs=2, space="PSUM") as ps_scan,
            tc.tile_pool(name="ps_out", bufs=2, space="PSUM") as ps_out,
        ):
            # load order: scan weights + x first (critical path) on the sync
            # ring; index data + lookback weights on the scalar ring.
            lt_sb = const_pool.tile([BLK, NBLK * BLK], F16, tag="lt")
            nc.sync.dma_start(lt_sb[:], lt[:])
            x_sb = const_pool.tile([BLK, NBLK * DIM], F16, tag="x")
            # stage x in pieces so early scan blocks unblock fast
            xcuts = sorted({min(1, NBLK), min(2, NBLK), min(3, NBLK),
                            min(5, NBLK), min(7, NBLK), NBLK})
            c_prev = 0
            for c1 in xcuts:
                if c1 <= c_prev:
                    continue
                nc.sync.dma_start(
                    x_sb[:, c_prev * DIM:c1 * DIM],
                    x[c_prev * BLK:c1 * BLK, :].rearrange(
                        "(t p) d -> p t d", p=BLK),
                )
                c_prev = c1
            cidb_sb = const_pool.tile([BLK, SEQ], F16, tag="cidb")
            nc.scalar.dma_start(cidb_sb[:, :2048], cidb[:, :2048])
            nc.scalar.dma_start(cidb_sb[:, 2048:], cidb[:, 2048:])
            lt2_sb = const_pool.tile([BLK, NBLK * BLK], F16, tag="lt2")
            nc.scalar.dma_start(lt2_sb[:], lt2[:])

            # per-partition compare values: col k holds p + 128k (f32 — the
            # tensor_scalar scalar operand must be float32 for is_equal)
            iota8 = const_pool.tile([BLK, 3], F32, tag="iota8")
            nc.gpsimd.iota(iota8[:], [[BLK, 3]], channel_multiplier=1,
                           allow_small_or_imprecise_dtypes=True)

            # ema: one extra zeroed block so window stitches that read past
            # the last scan block pull zeros (sel never selects them)
            ema = const_pool.tile([BLK, (NBLK + 1) * DIM], F16, tag="ema")
            if need_zblk:
                nc.vector.memset(ema[:, NBLK * DIM:], 0.0)
            win_sb = {}
            for w in prod_ws:
                win_sb[w] = const_pool.tile([BLK, DIM], F16, tag=f"win{w}",
                                            name=f"win_{w}")

            # PE warmup: zero-weight matmuls accumulating into block 0's
            # psum (add 0, cannot be dead-code-eliminated). ~3us of PE
            # activity releases the HAM clock throttle before real work.
            zw = const_pool.tile([BLK, BLK], F16, tag="zw")
            nc.vector.memset(zw[:], 0.0)
            zx = const_pool.tile([BLK, 512], F16, tag="zx")
            nc.vector.memset(zx[:], 0.0)
            ps0 = ps_scan.tile([BLK, DIM], F32, tag="ps")
            for k in range(4):
                for h in range(2):
                    nc.tensor.matmul(
                        ps0[:, h * 512:(h + 1) * 512], lhsT=zw[:], rhs=zx[:],
                        start=(k == 0), stop=False,
                    )

            # ---- dechunk emitter (interleaved with the scan so the PE
            # queue never stalls behind scan blocks waiting on late DMAs) ---
            state = {"tb": 0, "gi": 0}

            def emit_group(grp):
                gi = state["gi"]
                og = outpool.tile([BLK, grp * DIM], F16, tag=f"og{grp}",
                                  name=f"og_{gi}")
                for i in range(grp):
                    tb = state["tb"]
                    ws = wins[tb]
                    w0 = ws[0]
                    sels = []
                    for k, w in enumerate(ws):
                        sel = selpool.tile([BLK, BLK], F16, tag="sel",
                                           name=f"sel_{tb}_{k}")
                        nc.vector.tensor_scalar(
                            out=sel[:],
                            in0=cidb_sb[:, tb * BLK:(tb + 1) * BLK],
                            scalar1=iota8[:, k:k + 1],
                            scalar2=None,
                            op0=mybir.AluOpType.is_equal,
                        )
                        sels.append((sel, w))
                    po = ps_out.tile([BLK, DIM], F32, tag="po",
                                     name=f"po_{tb}")
                    for wi, (sel, w) in enumerate(sels):
                        for h in range(2):
                            if w % BLK == 0:
                                rsrc = ema[:, (w // BLK) * DIM + h * 512:
                                           (w // BLK) * DIM + (h + 1) * 512]
                            else:
                                rsrc = win_sb[w][:, h * 512:(h + 1) * 512]
                            nc.tensor.matmul(
                                po[:, h * 512:(h + 1) * 512],
                                lhsT=sel[:],
                                rhs=rsrc,
                                start=(wi == 0),
                                stop=(wi == len(sels) - 1),
                            )
                    dst = og[:, i * DIM:(i + 1) * DIM]
                    # 3/8 of the psum->og casts on DVE, 5/8 on ACT (DVE also
                    # builds sels; ACT only splits the ema casts)
                    if (tb % 8) in (0, 3, 6):
                        nc.vector.tensor_copy(out=dst, in_=po[:])
                    else:
                        nc.scalar.copy(out=dst, in_=po[:])
                    state["tb"] = tb + 1
                tb0 = state["tb"] - grp
                dma_eng = nc.scalar if (gi % 2) == 0 else nc.sync
                dma_eng.dma_start(
                    out[tb0 * BLK:state["tb"] * BLK, :].rearrange(
                        "(i p) d -> p i d", p=BLK
                    ),
                    og[:].rearrange("p (i d) -> p i d", d=DIM),
                )
                state["gi"] = gi + 1

            # a group is ready once the last scan block it depends on (via
            # direct windows or stitched ones) is written
            group_need = []
            tb = 0
            for grp in GRPS:
                group_need.append(
                    max(wneed(w) for t in range(tb, tb + grp)
                        for w in wins[t]))
                tb += grp

            # ---- blocked matmul scan over chunk blocks ----
            for t in range(NBLK):
                ps = ps0 if t == 0 else ps_scan.tile([BLK, DIM], F32, tag="ps")
                for h in range(2):
                    sl = slice(h * 512, (h + 1) * 512)
                    xsl = slice(t * DIM + h * 512, t * DIM + (h + 1) * 512)
                    nc.tensor.matmul(
                        ps[:, sl],
                        lhsT=lt_sb[:, t * BLK:(t + 1) * BLK],
                        rhs=x_sb[:, xsl],
                        start=(t != 0),
                        stop=(t == 0),
                    )
                    if t > 0:
                        lsl = slice((t - 1) * DIM + h * 512,
                                    (t - 1) * DIM + (h + 1) * 512)
                        nc.tensor.matmul(
                            ps[:, sl],
                            lhsT=lt2_sb[:, t * BLK:(t + 1) * BLK],
                            rhs=x_sb[:, lsl],
                            start=False,
                            stop=True,
                        )
                # psum -> fp16 ema, split across DVE and ACT
                nc.vector.tensor_copy(
                    out=ema[:, t * DIM:t * DIM + 512], in_=ps[:, :512]
                )
                nc.scalar.copy(
                    out=ema[:, t * DIM + 512:(t + 1) * DIM], in_=ps[:, 512:]
                )
                # stitch any 64-offset windows unblocked by this block's
                # CAST: two legal 64-partition f16 copies (fast on DVE/ACT;
                # GpSimd tensor ops cost ~3.6us each, do not use it)
                for w in prod_at.get(t, ()):
                    t0 = w // BLK
                    nc.vector.tensor_copy(
                        out=win_sb[w][0:64, :],
                        in_=ema[64:BLK, t0 * DIM:(t0 + 1) * DIM],
                    )
                    nc.scalar.copy(
                        out=win_sb[w][64:BLK, :],
                        in_=ema[0:64, (t0 + 1) * DIM:(t0 + 2) * DIM],
                    )
                while (state["gi"] < len(GRPS)
                       and group_need[state["gi"]] <= t):
                    emit_group(GRPS[state["gi"]])

            while state["gi"] < len(GRPS):
                emit_group(GRPS[state["gi"]])

    nc.finalize()
    return nc


def _run(in_maps, NBLK, wins, prod_ws):
    nc = _build_nc(NBLK, wins, prod_ws)
    res = run_bass_kernel_spmd(nc, in_maps, core_ids=list(range(NCORES)))
    return np.stack(
        [res.results[i]["out"].astype(np.float32) for i in range(NCORES)], axis=0
    )


def kernel(chunk_states, boundary_mask, boundary_prob):
    in_maps, NBLK, wins, prod_ws = _preprocess(
        chunk_states, boundary_mask, boundary_prob
    )
    last_err = None
    for _ in range(3):  # retry transient accelerator failures
        try:
            return _run(in_maps, NBLK, wins, prod_ws)
        except Exception as e:  # noqa: BLE001
            last_err = e
            try:
                import jax

                jax.clear_caches()
            except Exception:  # noqa: BLE001
                pass
    raise last_err
